# revision 1
# baseline (speedup 1.0000x reference)
"""CTC loss (log_softmax over time + CTC forward DP) on 8 Trainium2 NeuronCores.

Two SPMD launches:

Phase 1 (time-sharded): core c owns time slice [c*T/8, (c+1)*T/8) of ALL
batches. It streams its [B, T/8, C] slab in [128, C] tiles (2 batches x 64
timesteps per tile), gathers each batch's 33 unique label columns (32 targets
+ blank) with baked copies — indices are identical across cores because every
core sees every batch — exponentiates, computes per-(column,batch) partial
sumexp over its time slice (PE matmul with a per-batch selector), and writes
the exp'd gather (33 x B x T/8, ~0.3 MB) back to HBM via the ACT DMA ring
(separate FIFO from the input loads).

Host: sums partial sumexps into q[s,b] = e^c0 / sumexp (the log_softmax-over-
time denominator, expanded to the 65 extended states), reassembles the
gathered data, and redistributes it for phase 2: pairs of cores split each
batch group's time range in halves; the backward half gets s- and t-reversed
data so both directions run the same program.

Phase 2 (batch+direction sharded): the CTC forward recursion in probability
space is the linear recurrence E' = (A @ E) * W_t (plus A2 @ (E*kmask) when
adjacent repeated labels exist), with the banded transition as resident PE
weights and the W multiply one DVE op from PSUM. W is built on-chip by a
constant 33->65 expansion matmul scaled by q. Every RENORM_EVERY steps E is
renormalized by its column sum (PE sum -> reciprocal -> PE rank-1 broadcast
-> multiply) to stay in f32 range; the log corrections accumulate off the
critical chain. Host combines the forward/backward halves per batch in f64.
"""

from contextlib import ExitStack

import numpy as np

import concourse.bacc as bacc
import concourse.tile as tile
from concourse import mybir
from concourse.bass_utils import run_bass_kernel_spmd

BLANK = 6624
N_CORES = 8
C0 = 5.64  # per-step rescale folded into W
RENORM_EVERY = 16

F32 = mybir.dt.float32
F32R = mybir.dt.float32r

LAST_RESULTS = None  # (phase1 BassKernelResults, phase2 BassKernelResults)
_P1_CACHE = {}
_P2_CACHE = {}

Exp = mybir.ActivationFunctionType.Exp
Ln = mybir.ActivationFunctionType.Ln


def _build_phase1(b_tot, t_slice, c_dim, u_dim, ucols):
    """Gather + exp + partial sumexp for all batches over this core's time
    slice. ucols: [b_tot, u_dim] baked gather columns (identical across
    cores)."""
    bpt = min(max(1, 128 // t_slice), b_tot)
    assert bpt * t_slice <= 128, "time slice too large for one tile"
    assert b_tot % bpt == 0
    n_tiles = b_tot // bpt
    rows = bpt * t_slice

    nc = bacc.Bacc("TRN2", num_devices=N_CORES)
    lp_t = nc.dram_tensor("lp", [b_tot, t_slice, c_dim], F32, kind="ExternalInput")
    ident_t = nc.dram_tensor("ident", [128, 128], F32, kind="ExternalInput")
    sel_t = nc.dram_tensor("sel", [128, bpt], F32, kind="ExternalInput")
    egb_t = nc.dram_tensor("egb", [u_dim, b_tot, t_slice], F32, kind="ExternalOutput")
    sq_t = nc.dram_tensor("sq", [u_dim, b_tot], F32, kind="ExternalOutput")

    with tile.TileContext(nc) as tc, ExitStack() as ctx:
        consts = ctx.enter_context(tc.tile_pool(name="consts", bufs=1))
        lp_pool = ctx.enter_context(tc.tile_pool(name="lp", bufs=3))
        eg_pool = ctx.enter_context(tc.tile_pool(name="eg", bufs=3))
        st_pool = ctx.enter_context(tc.tile_pool(name="st", bufs=3))
        sqs_pool = ctx.enter_context(tc.tile_pool(name="sqs", bufs=1))

        ident_sb = consts.tile([128, 128], F32, tag="ident")
        nc.sync.dma_start(out=ident_sb[:], in_=ident_t[:])
        sel = consts.tile([128, bpt], F32, tag="sel")
        nc.sync.dma_start(out=sel[:], in_=sel_t[:])

        with (
            tc.tile_pool(name="psq", bufs=1, space="PSUM") as psq_pool,
            tc.tile_pool(name="tp", bufs=3, space="PSUM") as tp_pool,
        ):
            psum_q = psq_pool.tile([u_dim, b_tot], F32, tag="psq")
            for k in range(n_tiles):
                b0 = k * bpt
                lpt = lp_pool.tile([rows, c_dim], F32, tag="lpt")
                nc.sync.dma_start(
                    out=lpt[:],
                    in_=lp_t[b0 : b0 + bpt, :, :].rearrange("b t c -> (b t) c"),
                )
                gath = eg_pool.tile([rows, u_dim], F32, tag="gath")
                for h in range(bpt):
                    rsl = slice(h * t_slice, (h + 1) * t_slice)
                    for j in range(u_dim):
                        c = int(ucols[b0 + h, j])
                        nc.vector.tensor_copy(
                            out=gath[rsl, j : j + 1], in_=lpt[rsl, c : c + 1]
                        )
                eg = eg_pool.tile([rows, u_dim], F32, tag="eg")
                nc.scalar.activation(eg[:], gath[:], Exp)
                nc.tensor.matmul(
                    psum_q[:, b0 : b0 + bpt],
                    lhsT=eg[:],
                    rhs=sel[:],
                    start=True,
                    stop=True,
                )
                tp = tp_pool.tile([u_dim, rows], F32, tag="tp")
                nc.tensor.transpose(tp[:], eg[:], ident_sb[:])
                stg = st_pool.tile([u_dim, rows], F32, tag="stg")
                nc.vector.tensor_copy(stg[:], tp[:])
                # ACT's DMA ring: don't head-of-line block the lp loads on SP
                nc.scalar.dma_start(
                    out=egb_t[:, b0 : b0 + bpt, :].rearrange("s b t -> s (b t)"),
                    in_=stg[:],
                )
            sqs = sqs_pool.tile([u_dim, b_tot], F32, tag="sqs")
            nc.vector.tensor_copy(sqs[:], psum_q[:])
        nc.sync.dma_start(out=sq_t[:], in_=sqs[:])
    nc.finalize()
    return nc


def _build_phase2(bc, t_steps, s_dim, u_dim, use_a2):
    """The DP. All per-core differences are input data."""
    nc = bacc.Bacc("TRN2", num_devices=N_CORES)
    egb_t = nc.dram_tensor("egb", [u_dim, bc, t_steps], F32, kind="ExternalInput")
    q_t = nc.dram_tensor("q", [s_dim, bc], F32, kind="ExternalInput")
    expt_t = nc.dram_tensor("expt", [u_dim, s_dim], F32, kind="ExternalInput")
    a0t_t = nc.dram_tensor("a0t", [s_dim, s_dim], F32R, kind="ExternalInput")
    if use_a2:
        a2t_t = nc.dram_tensor("a2t", [s_dim, s_dim], F32R, kind="ExternalInput")
        km_t = nc.dram_tensor("kmask", [s_dim, bc], F32, kind="ExternalInput")
    init_t = nc.dram_tensor("init", [s_dim, bc], F32, kind="ExternalInput")
    ones_t = nc.dram_tensor("ones_s", [s_dim, 1], F32R, kind="ExternalInput")
    efin_t = nc.dram_tensor("efin", [s_dim, bc], F32, kind="ExternalOutput")
    lacc_t = nc.dram_tensor("lacc", [1, bc], F32, kind="ExternalOutput")

    with tile.TileContext(nc) as tc, ExitStack() as ctx:
        consts = ctx.enter_context(tc.tile_pool(name="consts", bufs=1))
        w_pool = ctx.enter_context(tc.tile_pool(name="w", bufs=1))
        e_pool = ctx.enter_context(tc.tile_pool(name="e", bufs=3))
        sm_pool = ctx.enter_context(tc.tile_pool(name="sm", bufs=2))
        out_pool = ctx.enter_context(tc.tile_pool(name="out", bufs=1))

        a0t_sb = consts.tile([s_dim, s_dim], F32R, tag="a0t")
        nc.sync.dma_start(out=a0t_sb[:], in_=a0t_t[:])
        if use_a2:
            a2t_sb = consts.tile([s_dim, s_dim], F32R, tag="a2t")
            nc.sync.dma_start(out=a2t_sb[:], in_=a2t_t[:])
            km_sb = consts.tile([s_dim, bc], F32, tag="km")
            nc.sync.dma_start(out=km_sb[:], in_=km_t[:])
            ek_pool = ctx.enter_context(tc.tile_pool(name="ek", bufs=2))
        init_sb = consts.tile([s_dim, bc], F32, tag="init")
        nc.sync.dma_start(out=init_sb[:], in_=init_t[:])
        q_sb = consts.tile([s_dim, bc], F32, tag="q")
        nc.sync.dma_start(out=q_sb[:], in_=q_t[:])
        expt_sb = consts.tile([u_dim, s_dim], F32, tag="expt")
        nc.sync.dma_start(out=expt_sb[:], in_=expt_t[:])
        ones_s = consts.tile([s_dim, 1], F32R, tag="ones_s")
        nc.sync.dma_start(out=ones_s[:], in_=ones_t[:])
        ones_row = consts.tile([1, s_dim], F32, tag="ones_row")
        nc.vector.memset(ones_row[:], 1.0)

        egb_sb = w_pool.tile([u_dim, bc, t_steps], F32, tag="egb")
        nc.sync.dma_start(out=egb_sb[:], in_=egb_t[:])
        warr = w_pool.tile([s_dim, bc, t_steps], F32, tag="warr")
        with tc.tile_pool(name="wx", bufs=2, space="PSUM") as wx_pool:
            for b in range(bc):
                wx = wx_pool.tile([s_dim, t_steps], F32, tag="wx")
                nc.tensor.matmul(
                    wx[:], lhsT=expt_sb[:], rhs=egb_sb[:, b, :], start=True, stop=True
                )
                nc.vector.tensor_scalar_mul(
                    warr[:, b, :], in0=wx[:], scalar1=q_sb[:, b : b + 1]
                )

        with (
            tc.tile_pool(name="p1", bufs=2, space="PSUM") as p_pool,
            tc.tile_pool(name="rs", bufs=2, space="PSUM") as rs_pool,
            tc.tile_pool(name="pb", bufs=2, space="PSUM") as pb_pool,
        ):
            E = e_pool.tile([s_dim, bc], F32R, tag="E")
            nc.vector.tensor_mul(E[:], init_sb[:], warr[:, :, 0])
            logacc = sm_pool.tile([1, bc], F32, tag="lg")
            nc.vector.memset(logacc[:], 0.0)

            for t in range(1, t_steps):
                p1 = p_pool.tile([s_dim, bc], F32, tag="p1")
                if use_a2:
                    ek = ek_pool.tile([s_dim, bc], F32R, tag="EK")
                    nc.vector.tensor_mul(ek[:], E[:], km_sb[:])
                    nc.tensor.matmul(
                        p1[:], lhsT=a0t_sb[:], rhs=E[:], start=True, stop=False
                    )
                    nc.tensor.matmul(
                        p1[:], lhsT=a2t_sb[:], rhs=ek[:], start=False, stop=True
                    )
                else:
                    nc.tensor.matmul(
                        p1[:], lhsT=a0t_sb[:], rhs=E[:], start=True, stop=True
                    )
                En = e_pool.tile([s_dim, bc], F32R, tag="E")
                nc.vector.tensor_mul(En[:], p1[:], warr[:, :, t])
                E = En

                if t % RENORM_EVERY == 0:
                    ps = rs_pool.tile([1, bc], F32, tag="ps")
                    nc.tensor.matmul(
                        ps[:], lhsT=ones_s[:], rhs=E[:], start=True, stop=True
                    )
                    rr = sm_pool.tile([1, bc], F32, tag="rr")
                    nc.vector.reciprocal(rr[:], ps[:])
                    # log correction runs off the serial chain
                    lnv = sm_pool.tile([1, bc], F32, tag="lnv")
                    nc.scalar.activation(lnv[:], ps[:], Ln)
                    lg2 = sm_pool.tile([1, bc], F32, tag="lg")
                    nc.vector.tensor_add(lg2[:], logacc[:], lnv[:])
                    logacc = lg2
                    pb = pb_pool.tile([s_dim, bc], F32, tag="pb")
                    nc.tensor.matmul(
                        pb[:], lhsT=ones_row[:], rhs=rr[:], start=True, stop=True
                    )
                    En2 = e_pool.tile([s_dim, bc], F32R, tag="E")
                    nc.vector.tensor_mul(En2[:], E[:], pb[:])
                    E = En2

            sv = out_pool.tile([s_dim, bc], F32, tag="sv")
            nc.vector.tensor_copy(sv[:], E[:])
            svl = out_pool.tile([1, bc], F32, tag="svl")
            nc.vector.tensor_copy(svl[:], logacc[:])
            nc.sync.dma_start(out=efin_t[:], in_=sv[:])
            nc.sync.dma_start(out=lacc_t[:], in_=svl[:])
    nc.finalize()
    return nc


def kernel(log_probs, targets, input_lengths, target_lengths):
    global LAST_RESULTS
    log_probs = np.asarray(log_probs, dtype=np.float32)
    tgt = np.asarray(targets).astype(np.int64)
    ilen = np.asarray(input_lengths).astype(np.int64)
    tlen = np.asarray(target_lengths).astype(np.int64)
    b_tot, t_len, c_dim = log_probs.shape
    l_max = tgt.shape[1]
    s_dim = 2 * l_max + 1
    u_dim = l_max + 1  # unique columns: labels + blank
    n_pairs = N_CORES // 2
    assert b_tot % n_pairs == 0
    bc = b_tot // n_pairs
    assert t_len % (2 * N_CORES) == 0
    t_slice = t_len // N_CORES
    t_half = t_len // 2
    assert (ilen == t_len).all(), "variable input_lengths not supported"

    ucols = np.concatenate(
        [tgt, np.full((b_tot, 1), BLANK, dtype=np.int64)], axis=1
    )  # [b, u]

    ext = np.full((b_tot, s_dim), BLANK, dtype=np.int64)
    ext[:, 1::2] = tgt
    ext_m2 = np.full_like(ext, BLANK)
    ext_m2[:, 2:] = ext[:, :-2]
    allow_skip = (ext != BLANK) & (ext != ext_m2)  # [b, s]

    # collisions among real labels force the two-matmul general path
    coll = False
    for b in range(b_tot):
        for s in range(3, min(2 * int(tlen[b]) + 1, s_dim), 2):
            if not allow_skip[b, s]:
                coll = True
    use_a2 = bool(coll)

    # s -> unique column map (same for every batch)
    smap = np.zeros(s_dim, dtype=np.int64)
    smap[0::2] = l_max
    smap[1::2] = np.arange(l_max)

    # ---- phase 1 ----
    key1 = (b_tot, t_slice, c_dim, u_dim, ucols.tobytes())
    if key1 not in _P1_CACHE:
        _P1_CACHE.clear()
        _P1_CACHE[key1] = _build_phase1(b_tot, t_slice, c_dim, u_dim, ucols)
    nc1 = _P1_CACHE[key1]

    ident = np.eye(128, dtype=np.float32)
    bpt = min(max(1, 128 // t_slice), b_tot)
    sel_np = np.zeros((128, bpt), dtype=np.float32)
    for h in range(bpt):
        sel_np[h * t_slice : (h + 1) * t_slice, h] = 1.0
    in_maps1 = []
    for c in range(N_CORES):
        sl = np.ascontiguousarray(log_probs[:, c * t_slice : (c + 1) * t_slice, :])
        in_maps1.append({"lp": sl, "ident": ident, "sel": sel_np})
    res1 = run_bass_kernel_spmd(nc1, in_maps1, list(range(N_CORES)))

    sumexp = np.zeros((u_dim, b_tot), dtype=np.float64)
    egb_full = np.zeros((u_dim, b_tot, t_len), dtype=np.float32)
    for c in range(N_CORES):
        sumexp += res1.results[c]["sq"].astype(np.float64)
        egb_full[:, :, c * t_slice : (c + 1) * t_slice] = res1.results[c]["egb"]
    q65_full = (np.exp(C0) / sumexp[smap, :]).astype(np.float32)  # [s, b]

    # ---- phase 2 ----
    key2 = (bc, t_half, s_dim, u_dim, use_a2)
    if key2 not in _P2_CACHE:
        _P2_CACHE.clear()
        _P2_CACHE[key2] = _build_phase2(bc, t_half, s_dim, u_dim, use_a2)
    nc2 = _P2_CACHE[key2]

    # expansion matrices (fwd: s -> smap[s]; bwd: s-reversed)
    expt_f = np.zeros((u_dim, s_dim), dtype=np.float32)
    expt_f[smap, np.arange(s_dim)] = 1.0
    expt_b = np.zeros((u_dim, s_dim), dtype=np.float32)
    expt_b[smap[::-1], np.arange(s_dim)] = 1.0

    a0 = np.eye(s_dim, dtype=np.float64) + np.eye(s_dim, k=-1, dtype=np.float64)
    if use_a2:
        a0t_f = a0t_b = np.ascontiguousarray(a0.T).astype(np.float32)
        a2 = np.eye(s_dim, k=-2, dtype=np.float32)
        a2t = np.ascontiguousarray(a2.T)
    else:
        # fold the skip pattern (all odd states) into a single matrix
        acomb = a0.copy()
        for s in range(3, s_dim, 2):
            acomb[s, s - 2] = 1.0
        # backward: J A^T J has the same banded form with K~[s] = K[s_dim+1-s];
        # for the all-odd pattern K~ hits odd s too (s_dim odd => parity kept)
        a0t_f = np.ascontiguousarray(acomb.T).astype(np.float32)
        a0t_b = a0t_f  # symmetric pattern: K~[s]=K[s_dim+1-s], odd->odd
    in_maps2 = []
    for c in range(N_CORES):
        pair = c // 2
        fwd = c % 2 == 0
        bs = slice(pair * bc, (pair + 1) * bc)
        init = np.zeros((s_dim, bc), dtype=np.float32)
        km = np.zeros((s_dim, bc), dtype=np.float32)
        if fwd:
            egb = np.ascontiguousarray(egb_full[:, bs, :t_half])
            qv = np.ascontiguousarray(q65_full[:, bs])
            expt = expt_f
            a0t_c = a0t_f
            km[: s_dim - 2, :] = allow_skip[bs, 2:].T
            init[0, :] = 1.0
            init[1, :] = 1.0
        else:
            egb = np.ascontiguousarray(egb_full[:, bs, : t_half - 1 : -1])
            qv = np.ascontiguousarray(q65_full[::-1, bs])
            expt = expt_b
            a0t_c = a0t_b
            for bi, bg in enumerate(range(pair * bc, (pair + 1) * bc)):
                for u in range(s_dim - 2):
                    km[u, bi] = allow_skip[bg, s_dim - 1 - u]
                lb = int(tlen[bg])
                i1 = 2 * lb
                i2 = max(2 * lb - 1, 0)
                init[s_dim - 1 - i1, bi] = 1.0
                init[s_dim - 1 - i2, bi] += 1.0
        m = {"egb": egb, "q": qv, "expt": expt, "a0t": a0t_c, "init": init,
             "ones_s": np.ones((s_dim, 1), dtype=np.float32)}
        if use_a2:
            m["a2t"] = a2t
            m["kmask"] = km
        in_maps2.append(m)
    res2 = run_bass_kernel_spmd(nc2, in_maps2, list(range(N_CORES)))
    LAST_RESULTS = (res1, res2)

    # ---- host combine (float64) ----
    losses = np.zeros(b_tot, dtype=np.float64)
    for pair in range(n_pairs):
        cf, cb = 2 * pair, 2 * pair + 1
        ef = res2.results[cf]["efin"].astype(np.float64)
        lf = res2.results[cf]["lacc"].astype(np.float64)[0]
        eb = res2.results[cb]["efin"].astype(np.float64)
        lb_ = res2.results[cb]["lacc"].astype(np.float64)[0]
        for bi in range(bc):
            bg = pair * bc + bi
            y = eb[::-1, bi]
            ab = np.eye(s_dim) + np.eye(s_dim, k=-1)
            for s in range(2, s_dim):
                if allow_skip[bg, s]:
                    ab[s, s - 2] = 1.0
            u = ab.T @ y
            val = float(u @ ef[:, bi])
            lam = lf[bi] + lb_[bi]
            if not np.isfinite(val) or val <= 0.0:
                loss = np.inf
            else:
                loss = -(np.log(val) - t_len * C0 + lam)
            if loss > 1e20:
                loss = 0.0  # zero_infinity
            losses[bg] = loss / max(int(tlen[bg]), 1)
    return np.float32(losses.mean())



# revision 3
# speedup vs baseline: 3.1897x; 3.1897x over previous
"""CTC loss (log_softmax over time + CTC forward DP) on 8 Trainium2 NeuronCores.

Only 33 of the 6625 vocabulary columns are ever used per batch (32 targets +
blank), and both the log_softmax-over-time denominator and the DP emissions
depend on those columns alone. Phase 1 therefore gathers just those elements
(~68k per core) with gpsimd indirect DMA — offsets are data, so the SPMD
program stays identical across cores while the batch shard differs — then
exponentiates on ACT and time-reduces on DVE. This replaces streaming the
full 434 MB input.

Phase 2 reformulates the CTC forward recursion as a wavefront over the 65
extended states: for fixed state s, alpha[t,s] = (alpha[t-1,s] + c[t]) * w[t,s]
with c[t] = alpha[t-1,s-1] + km[s]*alpha[t-1,s-2] is a first-order linear
recurrence in t — exactly the DVE's tensor_tensor_scan (state = (data0 + state)
* data1). 65 scans (+33 small builds) replace 512 sequential matmul steps.
Four batches per core with forward/backward time halves as 8 scan lanes; host
combines the halves in f64 (the classic fwd/bwd bridge). A global per-step
rescale e^{C0} keeps the f32 probability-domain scan in range (measured:
per-batch drift +-26 e-units, ridge excursions +49; C0=5.80 centers both
inside f32 exponent range).
"""

from contextlib import ExitStack

import numpy as np

import concourse.bacc as bacc
import concourse.bass as bass
import concourse.tile as tile
from concourse import mybir
from concourse.bass_utils import run_bass_kernel_spmd

BLANK = 6624
N_CORES = 8
C0 = 5.80

F32 = mybir.dt.float32
I32 = mybir.dt.int32

LAST_RESULTS = None  # (phase1 BassKernelResults, phase2 BassKernelResults)
_P1_CACHE = {}
_P2_CACHE = {}

Exp = mybir.ActivationFunctionType.Exp
Add = mybir.AluOpType.add
Mult = mybir.AluOpType.mult


def _build_phase1(bc, t_len, c_dim, u_dim):
    """Indirect-DMA gather of each batch's u_dim label columns (full time
    range) + exp + time sums. Offsets come in as data, so the program is
    identical across cores."""
    nc = bacc.Bacc("TRN2", num_devices=N_CORES)
    lp_t = nc.dram_tensor("lp", [bc, t_len, c_dim], F32, kind="ExternalInput")
    idx_t = nc.dram_tensor("idx", [u_dim, bc * t_len], I32, kind="ExternalInput")
    egb_t = nc.dram_tensor("egb", [u_dim, bc * t_len], F32, kind="ExternalOutput")
    sq_t = nc.dram_tensor("sq", [u_dim, bc], F32, kind="ExternalOutput")

    with tile.TileContext(nc) as tc, ExitStack() as ctx:
        consts = ctx.enter_context(tc.tile_pool(name="consts", bufs=1))
        eg_pool = ctx.enter_context(tc.tile_pool(name="eg", bufs=2))
        ege_pool = ctx.enter_context(tc.tile_pool(name="ege", bufs=2))
        sq_pool = ctx.enter_context(tc.tile_pool(name="sq", bufs=1))

        idx_sb = consts.tile([u_dim, bc * t_len], I32, tag="idx")
        nc.sync.dma_start(out=idx_sb[:], in_=idx_t[:])
        sq_sb = sq_pool.tile([u_dim, bc], F32, tag="sq")

        # chunk by local batch: pipelines Pool descriptor generation with the
        # DMA transfer and the ACT/DVE tail
        for b in range(bc):
            sl = slice(b * t_len, (b + 1) * t_len)
            eg = eg_pool.tile([u_dim, t_len], F32, tag="eg")
            nc.gpsimd.indirect_dma_start(
                out=eg[:],
                out_offset=None,
                in_=lp_t[:],
                in_offset=bass.IndirectOffsetOnAxis(ap=idx_sb[:, sl], axis=2),
            )
            ege = ege_pool.tile([u_dim, t_len], F32, tag="ege")
            nc.scalar.activation(ege[:], eg[:], Exp)
            nc.vector.tensor_reduce(
                out=sq_sb[:, b : b + 1],
                in_=ege[:],
                axis=mybir.AxisListType.X,
                op=Add,
            )
            nc.scalar.dma_start(out=egb_t[:, sl], in_=ege[:])
        nc.sync.dma_start(out=sq_t[:], in_=sq_sb[:])
    nc.finalize()
    return nc


def _build_phase2(lanes, s_dim, t_half):
    """The DP: one tensor_tensor_scan per extended state row.

    W rows (emission probs * q, time-reversed for bwd lanes) and km (skip
    masks) are inputs, so the program is identical across cores."""
    n_ev = (s_dim + 1) // 2  # even rows s=0,2,...
    n_od = s_dim // 2  # odd rows

    nc = bacc.Bacc("TRN2", num_devices=N_CORES)
    wev_t = nc.dram_tensor("wev", [lanes, n_ev, t_half], F32, kind="ExternalInput")
    wod_t = nc.dram_tensor("wod", [lanes, n_od, t_half], F32, kind="ExternalInput")
    km_t = nc.dram_tensor("km", [lanes, s_dim], F32, kind="ExternalInput")
    efev_t = nc.dram_tensor("efev", [lanes, n_ev], F32, kind="ExternalOutput")
    efod_t = nc.dram_tensor("efod", [lanes, n_od], F32, kind="ExternalOutput")

    with tile.TileContext(nc) as tc, ExitStack() as ctx:
        consts = ctx.enter_context(tc.tile_pool(name="consts", bufs=1))
        a_pool = ctx.enter_context(tc.tile_pool(name="a", bufs=1))
        c_pool = ctx.enter_context(tc.tile_pool(name="c", bufs=2))
        out_pool = ctx.enter_context(tc.tile_pool(name="out", bufs=1))

        wev = consts.tile([lanes, n_ev, t_half], F32, tag="wev")
        nc.sync.dma_start(out=wev[:], in_=wev_t[:])
        wod = consts.tile([lanes, n_od, t_half], F32, tag="wod")
        nc.sync.dma_start(out=wod[:], in_=wod_t[:])
        km = consts.tile([lanes, s_dim], F32, tag="km")
        nc.sync.dma_start(out=km[:], in_=km_t[:])

        onehot = consts.tile([lanes, t_half], F32, tag="onehot")
        nc.vector.memset(onehot[:], 0.0)
        nc.vector.memset(onehot[:, 0:1], 1.0)

        # alpha rows with a zero guard column at t=0 (col 0); scans write 1..
        aev = a_pool.tile([lanes, n_ev, t_half + 1], F32, tag="aev")
        aod = a_pool.tile([lanes, n_od, t_half + 1], F32, tag="aod")
        nc.vector.memset(aev[:, :, 0:1], 0.0)
        nc.vector.memset(aod[:, :, 0:1], 0.0)

        for s in range(s_dim):
            j = s // 2
            if s == 0:
                d0 = onehot[:]
            elif s == 1:
                c = c_pool.tile([lanes, t_half], F32, tag="c")
                nc.vector.tensor_add(c[:], onehot[:], aev[:, 0, 0:t_half])
                d0 = c[:]
            elif s % 2 == 0:
                d0 = aod[:, j - 1, 0:t_half]
            else:
                c = c_pool.tile([lanes, t_half], F32, tag="c")
                nc.vector.scalar_tensor_tensor(
                    out=c[:],
                    in0=aod[:, j - 1, 0:t_half],
                    scalar=km[:, s : s + 1],
                    in1=aev[:, j, 0:t_half],
                    op0=Mult,
                    op1=Add,
                )
                d0 = c[:]
            if s % 2 == 0:
                w_s = wev[:, j, :]
                out_s = aev[:, j, 1 : t_half + 1]
            else:
                w_s = wod[:, j, :]
                out_s = aod[:, j, 1 : t_half + 1]
            nc.vector.tensor_tensor_scan(
                out=out_s, data0=d0, data1=w_s, initial=0.0, op0=Add, op1=Mult
            )

        efev = out_pool.tile([lanes, n_ev], F32, tag="efev")
        nc.vector.tensor_copy(
            efev[:], aev[:, :, t_half : t_half + 1].rearrange("p a b -> p (a b)")
        )
        efod = out_pool.tile([lanes, n_od], F32, tag="efod")
        nc.vector.tensor_copy(
            efod[:], aod[:, :, t_half : t_half + 1].rearrange("p a b -> p (a b)")
        )
        nc.sync.dma_start(out=efev_t[:], in_=efev[:])
        nc.sync.dma_start(out=efod_t[:], in_=efod[:])
    nc.finalize()
    return nc


def kernel(log_probs, targets, input_lengths, target_lengths):
    global LAST_RESULTS
    log_probs = np.asarray(log_probs, dtype=np.float32)
    tgt = np.asarray(targets).astype(np.int64)
    ilen = np.asarray(input_lengths).astype(np.int64)
    tlen = np.asarray(target_lengths).astype(np.int64)
    b_tot, t_len, c_dim = log_probs.shape
    l_max = tgt.shape[1]
    s_dim = 2 * l_max + 1
    u_dim = l_max + 1
    t_half = t_len // 2
    bc = b_tot // N_CORES  # batches per core
    lanes = 2 * bc  # fwd + bwd
    assert (ilen == t_len).all(), "variable input_lengths not supported"
    assert (tlen == l_max).all(), "variable target_lengths not supported"

    ucols = np.concatenate(
        [tgt, np.full((b_tot, 1), BLANK, dtype=np.int64)], axis=1
    )  # [b, u]
    smap = np.zeros(s_dim, dtype=np.int64)
    smap[0::2] = l_max
    smap[1::2] = np.arange(l_max)

    ext = np.full((b_tot, s_dim), BLANK, dtype=np.int64)
    ext[:, 1::2] = tgt
    ext_m2 = np.full_like(ext, BLANK)
    ext_m2[:, 2:] = ext[:, :-2]
    allow_skip = (ext != BLANK) & (ext != ext_m2)  # [b, s]

    # ---- phase 1: batch-sharded indirect gather + exp + time sums ----
    key1 = (bc, t_len, c_dim, u_dim)
    if key1 not in _P1_CACHE:
        _P1_CACHE.clear()
        _P1_CACHE[key1] = _build_phase1(bc, t_len, c_dim, u_dim)
    nc1 = _P1_CACHE[key1]

    # flat element offsets into the [bc, t_len, c_dim] slab
    base = (np.arange(bc)[:, None] * t_len + np.arange(t_len)[None, :]) * c_dim
    in_maps1 = []
    for c in range(N_CORES):
        bs = slice(c * bc, (c + 1) * bc)
        idx = (base[None, :, :] + ucols[bs].T[:, :, None]).reshape(
            u_dim, bc * t_len
        )
        in_maps1.append(
            {
                "lp": log_probs[bs],
                "idx": np.ascontiguousarray(idx.astype(np.int32)),
            }
        )
    res1 = run_bass_kernel_spmd(nc1, in_maps1, list(range(N_CORES)))

    sumexp = np.zeros((u_dim, b_tot), dtype=np.float64)
    egb_full = np.zeros((u_dim, b_tot, t_len), dtype=np.float32)
    for c in range(N_CORES):
        bs = slice(c * bc, (c + 1) * bc)
        sumexp[:, bs] = res1.results[c]["sq"].astype(np.float64)
        egb_full[:, bs, :] = res1.results[c]["egb"].reshape(u_dim, bc, t_len)
    q = (np.float32(np.exp(C0)) / sumexp.astype(np.float32)).astype(np.float32)
    egq = egb_full * q[:, :, None]  # [u, b, t] f32

    # ---- phase 2: batch-sharded scan DP ----
    key2 = (lanes, s_dim, t_half)
    if key2 not in _P2_CACHE:
        _P2_CACHE.clear()
        _P2_CACHE[key2] = _build_phase2(lanes, s_dim, t_half)
    nc2 = _P2_CACHE[key2]

    # per-state W rows: fwd row s ~ egq[smap[s], b, 0:Th]; bwd row s ~ state
    # (S-1)-s with reversed time (orig t = T-1-t')
    wf = egq[smap, :, 0:t_half]  # [s, b, th]
    wb = egq[smap[::-1], :, ::-1][:, :, 0:t_half]  # [s, b, th]
    kmf = np.zeros((b_tot, s_dim), dtype=np.float32)
    kmf[:, 2:] = allow_skip[:, 2:].astype(np.float32)
    kmb = np.zeros((b_tot, s_dim), dtype=np.float32)
    for s in range(2, s_dim):
        kmb[:, s] = allow_skip[:, s_dim + 1 - s].astype(np.float32)

    in_maps2 = []
    for c in range(N_CORES):
        bs = slice(c * bc, (c + 1) * bc)
        wl = np.concatenate([wf[:, bs, :], wb[:, bs, :]], axis=1)  # [s, lanes, th]
        wl = np.ascontiguousarray(np.transpose(wl, (1, 0, 2)))  # [lanes, s, th]
        kml = np.concatenate([kmf[bs], kmb[bs]], axis=0)  # [lanes, s]
        in_maps2.append(
            {
                "wev": np.ascontiguousarray(wl[:, 0::2, :]),
                "wod": np.ascontiguousarray(wl[:, 1::2, :]),
                "km": np.ascontiguousarray(kml),
            }
        )
    res2 = run_bass_kernel_spmd(nc2, in_maps2, list(range(N_CORES)))
    LAST_RESULTS = (res1, res2)

    # ---- host combine (float64) ----
    losses = np.zeros(b_tot, dtype=np.float64)
    for c in range(N_CORES):
        efev = res2.results[c]["efev"].astype(np.float64)  # [lanes, n_ev]
        efod = res2.results[c]["efod"].astype(np.float64)
        ef_all = np.zeros((lanes, s_dim))
        ef_all[:, 0::2] = efev
        ef_all[:, 1::2] = efod
        for bi in range(bc):
            bg = c * bc + bi
            ef = ef_all[bi]  # fwd lane, index = state s
            y = ef_all[bc + bi][::-1]  # bwd lane, flipped to state order
            ab = np.eye(s_dim) + np.eye(s_dim, k=-1)
            for s in range(2, s_dim):
                if allow_skip[bg, s]:
                    ab[s, s - 2] = 1.0
            u = ab.T @ y
            val = float(u @ ef)
            if not np.isfinite(val) or val <= 0.0:
                loss = np.inf
            else:
                loss = -(np.log(val) - t_len * C0)
            if loss > 1e20:
                loss = 0.0  # zero_infinity
            losses[bg] = loss / max(int(tlen[bg]), 1)
    return np.float32(losses.mean())


# revision 4
# speedup vs baseline: 3.4109x; 1.0693x over previous
"""CTC loss (log_softmax over time + CTC forward DP) on 8 Trainium2 NeuronCores.

Only 33 of the 6625 vocabulary columns are ever used per batch (32 targets +
blank), and both the log_softmax-over-time denominator and the DP emissions
depend on those columns alone. Phase 1 therefore gathers just those elements
(~68k per core) with gpsimd indirect DMA — offsets are data, so the SPMD
program stays identical across cores while the batch shard differs — then
exponentiates on ACT and time-reduces on DVE. This replaces streaming the
full 434 MB input.

Phase 2 reformulates the CTC forward recursion as a wavefront over the 65
extended states: for fixed state s, alpha[t,s] = (alpha[t-1,s] + c[t]) * w[t,s]
with c[t] = alpha[t-1,s-1] + km[s]*alpha[t-1,s-2] is a first-order linear
recurrence in t — exactly the DVE's tensor_tensor_scan (state = (data0 + state)
* data1). 65 scans (+33 small builds) replace 512 sequential matmul steps.
Four batches per core with forward/backward time halves as 8 scan lanes; host
combines the halves in f64 (the classic fwd/bwd bridge). A global per-step
rescale e^{C0} keeps the f32 probability-domain scan in range (measured:
per-batch drift +-26 e-units, ridge excursions +49; C0=5.80 centers both
inside f32 exponent range).
"""

from contextlib import ExitStack

import numpy as np
from ml_dtypes import bfloat16 as _bf16

import concourse.bacc as bacc
import concourse.bass as bass
import concourse.tile as tile
from concourse import mybir
from concourse.bass_utils import run_bass_kernel_spmd

BLANK = 6624
N_CORES = 8
C0 = 5.80

F32 = mybir.dt.float32
BF16 = mybir.dt.bfloat16
I32 = mybir.dt.int32

LAST_RESULTS = None  # (phase1 BassKernelResults, phase2 BassKernelResults)
_P1_CACHE = {}
_P2_CACHE = {}

Exp = mybir.ActivationFunctionType.Exp
Add = mybir.AluOpType.add
Mult = mybir.AluOpType.mult


def _build_phase1(bc, t_len, c_dim, u_dim):
    """Indirect-DMA gather of each batch's u_dim label columns (full time
    range) + exp + time sums. Offsets come in as data, so the program is
    identical across cores."""
    nc = bacc.Bacc("TRN2", num_devices=N_CORES)
    lp_t = nc.dram_tensor("lp", [bc, t_len, c_dim], F32, kind="ExternalInput")
    idx_t = nc.dram_tensor("idx", [u_dim, bc * t_len], I32, kind="ExternalInput")
    egb_t = nc.dram_tensor("egb", [u_dim, bc * t_len], F32, kind="ExternalOutput")
    sq_t = nc.dram_tensor("sq", [u_dim, bc], F32, kind="ExternalOutput")

    with tile.TileContext(nc) as tc, ExitStack() as ctx:
        consts = ctx.enter_context(tc.tile_pool(name="consts", bufs=1))
        eg_pool = ctx.enter_context(tc.tile_pool(name="eg", bufs=2))
        ege_pool = ctx.enter_context(tc.tile_pool(name="ege", bufs=2))
        sq_pool = ctx.enter_context(tc.tile_pool(name="sq", bufs=1))

        idx_sb = consts.tile([u_dim, bc * t_len], I32, tag="idx")
        nc.sync.dma_start(out=idx_sb[:], in_=idx_t[:])
        sq_sb = sq_pool.tile([u_dim, bc], F32, tag="sq")

        # chunk by local batch: pipelines Pool descriptor generation with the
        # DMA transfer and the ACT/DVE tail
        for b in range(bc):
            sl = slice(b * t_len, (b + 1) * t_len)
            eg = eg_pool.tile([u_dim, t_len], F32, tag="eg")
            nc.gpsimd.indirect_dma_start(
                out=eg[:],
                out_offset=None,
                in_=lp_t[:],
                in_offset=bass.IndirectOffsetOnAxis(ap=idx_sb[:, sl], axis=2),
            )
            ege = ege_pool.tile([u_dim, t_len], F32, tag="ege")
            nc.scalar.activation(ege[:], eg[:], Exp)
            nc.vector.tensor_reduce(
                out=sq_sb[:, b : b + 1],
                in_=ege[:],
                axis=mybir.AxisListType.X,
                op=Add,
            )
            nc.scalar.dma_start(out=egb_t[:, sl], in_=ege[:])
        nc.sync.dma_start(out=sq_t[:], in_=sq_sb[:])
    nc.finalize()
    return nc


def _build_phase2(lanes, s_dim, t_half):
    """The DP: one tensor_tensor_scan per extended state row.

    W rows (emission probs * q, time-reversed for bwd lanes) and km (skip
    masks) are inputs, so the program is identical across cores."""
    n_ev = (s_dim + 1) // 2  # even rows s=0,2,...
    n_od = s_dim // 2  # odd rows

    nc = bacc.Bacc("TRN2", num_devices=N_CORES)
    wev_t = nc.dram_tensor("wev", [lanes, n_ev, t_half], BF16, kind="ExternalInput")
    wod_t = nc.dram_tensor("wod", [lanes, n_od, t_half], BF16, kind="ExternalInput")
    km_t = nc.dram_tensor("km", [lanes, s_dim], F32, kind="ExternalInput")
    efev_t = nc.dram_tensor("efev", [lanes, n_ev], F32, kind="ExternalOutput")
    efod_t = nc.dram_tensor("efod", [lanes, n_od], F32, kind="ExternalOutput")

    with tile.TileContext(nc) as tc, ExitStack() as ctx:
        consts = ctx.enter_context(tc.tile_pool(name="consts", bufs=1))
        a_pool = ctx.enter_context(tc.tile_pool(name="a", bufs=1))
        c_pool = ctx.enter_context(tc.tile_pool(name="c", bufs=2))
        out_pool = ctx.enter_context(tc.tile_pool(name="out", bufs=1))

        wev = consts.tile([lanes, n_ev, t_half], BF16, tag="wev")
        nc.sync.dma_start(out=wev[:], in_=wev_t[:])
        wod = consts.tile([lanes, n_od, t_half], BF16, tag="wod")
        nc.sync.dma_start(out=wod[:], in_=wod_t[:])
        km = consts.tile([lanes, s_dim], F32, tag="km")
        nc.sync.dma_start(out=km[:], in_=km_t[:])

        onehot = consts.tile([lanes, t_half], BF16, tag="onehot")
        nc.vector.memset(onehot[:], 0.0)
        nc.vector.memset(onehot[:, 0:1], 1.0)

        # alpha rows with a zero guard column at t=0 (col 0); scans write 1..
        aev = a_pool.tile([lanes, n_ev, t_half + 1], BF16, tag="aev")
        aod = a_pool.tile([lanes, n_od, t_half + 1], BF16, tag="aod")
        nc.vector.memset(aev[:, :, 0:1], 0.0)
        nc.vector.memset(aod[:, :, 0:1], 0.0)

        for s in range(s_dim):
            j = s // 2
            if s == 0:
                d0 = onehot[:]
            elif s == 1:
                c = c_pool.tile([lanes, t_half], BF16, tag="c")
                nc.vector.tensor_add(c[:], onehot[:], aev[:, 0, 0:t_half])
                d0 = c[:]
            elif s % 2 == 0:
                d0 = aod[:, j - 1, 0:t_half]
            else:
                c = c_pool.tile([lanes, t_half], BF16, tag="c")
                nc.vector.scalar_tensor_tensor(
                    out=c[:],
                    in0=aod[:, j - 1, 0:t_half],
                    scalar=km[:, s : s + 1],
                    in1=aev[:, j, 0:t_half],
                    op0=Mult,
                    op1=Add,
                )
                d0 = c[:]
            if s % 2 == 0:
                w_s = wev[:, j, :]
                out_s = aev[:, j, 1 : t_half + 1]
            else:
                w_s = wod[:, j, :]
                out_s = aod[:, j, 1 : t_half + 1]
            nc.vector.tensor_tensor_scan(
                out=out_s, data0=d0, data1=w_s, initial=0.0, op0=Add, op1=Mult
            )

        efev = out_pool.tile([lanes, n_ev], F32, tag="efev")
        nc.vector.tensor_copy(
            efev[:], aev[:, :, t_half : t_half + 1].rearrange("p a b -> p (a b)")
        )
        efod = out_pool.tile([lanes, n_od], F32, tag="efod")
        nc.vector.tensor_copy(
            efod[:], aod[:, :, t_half : t_half + 1].rearrange("p a b -> p (a b)")
        )
        nc.sync.dma_start(out=efev_t[:], in_=efev[:])
        nc.sync.dma_start(out=efod_t[:], in_=efod[:])
    nc.finalize()
    return nc


def kernel(log_probs, targets, input_lengths, target_lengths):
    global LAST_RESULTS
    log_probs = np.asarray(log_probs, dtype=np.float32)
    tgt = np.asarray(targets).astype(np.int64)
    ilen = np.asarray(input_lengths).astype(np.int64)
    tlen = np.asarray(target_lengths).astype(np.int64)
    b_tot, t_len, c_dim = log_probs.shape
    l_max = tgt.shape[1]
    s_dim = 2 * l_max + 1
    u_dim = l_max + 1
    t_half = t_len // 2
    bc = b_tot // N_CORES  # batches per core
    lanes = 2 * bc  # fwd + bwd
    assert (ilen == t_len).all(), "variable input_lengths not supported"
    assert (tlen == l_max).all(), "variable target_lengths not supported"

    ucols = np.concatenate(
        [tgt, np.full((b_tot, 1), BLANK, dtype=np.int64)], axis=1
    )  # [b, u]
    smap = np.zeros(s_dim, dtype=np.int64)
    smap[0::2] = l_max
    smap[1::2] = np.arange(l_max)

    ext = np.full((b_tot, s_dim), BLANK, dtype=np.int64)
    ext[:, 1::2] = tgt
    ext_m2 = np.full_like(ext, BLANK)
    ext_m2[:, 2:] = ext[:, :-2]
    allow_skip = (ext != BLANK) & (ext != ext_m2)  # [b, s]

    # ---- phase 1: batch-sharded indirect gather + exp + time sums ----
    key1 = (bc, t_len, c_dim, u_dim)
    if key1 not in _P1_CACHE:
        _P1_CACHE.clear()
        _P1_CACHE[key1] = _build_phase1(bc, t_len, c_dim, u_dim)
    nc1 = _P1_CACHE[key1]

    # flat element offsets into the [bc, t_len, c_dim] slab
    base = (np.arange(bc)[:, None] * t_len + np.arange(t_len)[None, :]) * c_dim
    in_maps1 = []
    for c in range(N_CORES):
        bs = slice(c * bc, (c + 1) * bc)
        idx = (base[None, :, :] + ucols[bs].T[:, :, None]).reshape(
            u_dim, bc * t_len
        )
        in_maps1.append(
            {
                "lp": log_probs[bs],
                "idx": np.ascontiguousarray(idx.astype(np.int32)),
            }
        )
    res1 = run_bass_kernel_spmd(nc1, in_maps1, list(range(N_CORES)))

    sumexp = np.zeros((u_dim, b_tot), dtype=np.float64)
    egb_full = np.zeros((u_dim, b_tot, t_len), dtype=np.float32)
    for c in range(N_CORES):
        bs = slice(c * bc, (c + 1) * bc)
        sumexp[:, bs] = res1.results[c]["sq"].astype(np.float64)
        egb_full[:, bs, :] = res1.results[c]["egb"].reshape(u_dim, bc, t_len)
    q = (np.float32(np.exp(C0)) / sumexp.astype(np.float32)).astype(np.float32)
    egq = egb_full * q[:, :, None]  # [u, b, t] f32

    # ---- phase 2: batch-sharded scan DP ----
    key2 = (lanes, s_dim, t_half)
    if key2 not in _P2_CACHE:
        _P2_CACHE.clear()
        _P2_CACHE[key2] = _build_phase2(lanes, s_dim, t_half)
    nc2 = _P2_CACHE[key2]

    # per-state W rows: fwd row s ~ egq[smap[s], b, 0:Th]; bwd row s ~ state
    # (S-1)-s with reversed time (orig t = T-1-t')
    wf = egq[smap, :, 0:t_half]  # [s, b, th]
    wb = egq[smap[::-1], :, ::-1][:, :, 0:t_half]  # [s, b, th]
    kmf = np.zeros((b_tot, s_dim), dtype=np.float32)
    kmf[:, 2:] = allow_skip[:, 2:].astype(np.float32)
    kmb = np.zeros((b_tot, s_dim), dtype=np.float32)
    for s in range(2, s_dim):
        kmb[:, s] = allow_skip[:, s_dim + 1 - s].astype(np.float32)

    in_maps2 = []
    for c in range(N_CORES):
        bs = slice(c * bc, (c + 1) * bc)
        wl = np.concatenate([wf[:, bs, :], wb[:, bs, :]], axis=1)  # [s, lanes, th]
        wl = np.ascontiguousarray(np.transpose(wl, (1, 0, 2)))  # [lanes, s, th]
        kml = np.concatenate([kmf[bs], kmb[bs]], axis=0)  # [lanes, s]
        in_maps2.append(
            {
                "wev": np.ascontiguousarray(wl[:, 0::2, :]).astype(_bf16),
                "wod": np.ascontiguousarray(wl[:, 1::2, :]).astype(_bf16),
                "km": np.ascontiguousarray(kml),
            }
        )
    res2 = run_bass_kernel_spmd(nc2, in_maps2, list(range(N_CORES)))
    LAST_RESULTS = (res1, res2)

    # ---- host combine (float64) ----
    losses = np.zeros(b_tot, dtype=np.float64)
    for c in range(N_CORES):
        efev = res2.results[c]["efev"].astype(np.float64)  # [lanes, n_ev]
        efod = res2.results[c]["efod"].astype(np.float64)
        ef_all = np.zeros((lanes, s_dim))
        ef_all[:, 0::2] = efev
        ef_all[:, 1::2] = efod
        for bi in range(bc):
            bg = c * bc + bi
            ef = ef_all[bi]  # fwd lane, index = state s
            y = ef_all[bc + bi][::-1]  # bwd lane, flipped to state order
            ab = np.eye(s_dim) + np.eye(s_dim, k=-1)
            for s in range(2, s_dim):
                if allow_skip[bg, s]:
                    ab[s, s - 2] = 1.0
            u = ab.T @ y
            val = float(u @ ef)
            if not np.isfinite(val) or val <= 0.0:
                loss = np.inf
            else:
                loss = -(np.log(val) - t_len * C0)
            if loss > 1e20:
                loss = 0.0  # zero_infinity
            losses[bg] = loss / max(int(tlen[bg]), 1)
    return np.float32(losses.mean())
